# revision 4
# baseline (speedup 1.0000x reference)
"""BertSelfAttention (B=2, S=2048, H=1024, 16 heads x 64) on 8 TRN2 NeuronCores.

Sharding: data parallel on batch (4 cores per batch) x tensor parallel on
heads (4 heads per core). No cross-core comms; each core computes
out[b, :, 256*g:256*(g+1)] for its head group g.

Per-core pipeline (all matmuls in float32r, ~2e-4 rel err):
  A) hidden [2048,1024] -> hiddenT tiles via PE transpose
  B) qT/kT [128(d of head pair), 2048(s)] projections (scale 1/8 and bias
     folded in), V [128(s), 4, 65] with a ones column per head (softmax
     denominator comes out of the ctx matmul for free)
  C) flash attention in scoresT layout: scoresT[k,q] matmuls (two heads
     row-packed on the PE, K=64 each), exp on ACT with the attention mask as
     per-partition bias, ctxT[65, q] accumulated over k chunks in PSUM
  D) PE-transpose ctxT -> [q, 65], DVE reciprocal + scale, DMA out
"""

import numpy as np

import concourse.bass as bass
import concourse.tile as tile
from concourse import bacc, mybir
from concourse.bass_utils import run_bass_kernel_spmd
from concourse.masks import make_identity

F32 = mybir.dt.float32
F32R = mybir.dt.float32r
EXP = mybir.ActivationFunctionType.Exp

B, S, H = 2, 2048, 1024
NH, HD = 16, 64
NCORES = 8
HPC = 4  # heads per core
DPC = HPC * HD  # 256 output dims per core
SC = S // 128  # 16 s/k chunks
JC = H // 128  # 8 contraction chunks
QB = 512  # q block in attention inner loop
NQB = S // QB


def build():
    nc = bacc.Bacc(
        "TRN2",
        target_bir_lowering=False,
        debug=False,
        enable_asserts=False,
        num_devices=NCORES,
    )
    hid = nc.dram_tensor("hid", [S, H], F32, kind="ExternalInput").ap()
    wq = nc.dram_tensor("wq", [H, DPC], F32, kind="ExternalInput").ap()
    wk = nc.dram_tensor("wk", [H, DPC], F32, kind="ExternalInput").ap()
    wv = nc.dram_tensor("wv", [H, DPC], F32, kind="ExternalInput").ap()
    bqs = nc.dram_tensor("bqs", [128, 2], F32, kind="ExternalInput").ap()
    bks = nc.dram_tensor("bks", [128, 2], F32, kind="ExternalInput").ap()
    bvs = nc.dram_tensor("bvs", [1, DPC], F32, kind="ExternalInput").ap()
    mask = nc.dram_tensor("mask", [128, SC], F32, kind="ExternalInput").ap()
    out = nc.dram_tensor("out", [S, DPC], F32, kind="ExternalOutput").ap()

    with tile.TileContext(nc) as tc:
        with tc.tile_pool(name="persist", bufs=1) as persist:
            ident = persist.tile([128, 128], F32, tag="ident")
            make_identity(nc, ident[:])
            ones1_f = persist.tile([1, 128], F32, tag="ones1f")
            nc.vector.memset(ones1_f[:], 1.0)
            ones1 = persist.tile([1, 128], F32R, tag="ones1")
            nc.vector.tensor_copy(ones1[:], ones1_f[:])
            ones4_f = persist.tile([128, HPC], F32, tag="ones4f")
            nc.vector.memset(ones4_f[:], 1.0)
            mask_sb = persist.tile([128, SC], F32, tag="mask")
            nc.sync.dma_start(mask_sb[:], mask)
            bqs_sb = persist.tile([128, 2], F32, tag="bqs")
            nc.sync.dma_start(bqs_sb[:], bqs)
            bks_sb = persist.tile([128, 2], F32, tag="bks")
            nc.sync.dma_start(bks_sb[:], bks)
            bvs_sb = persist.tile([1, DPC], F32R, tag="bvs")
            nc.sync.dma_start(bvs_sb[:], bvs.bitcast(F32R))

            # persistent activations
            qT = [persist.tile([128, S], F32R, tag=f"qT{p}", name=f"qT{p}") for p in range(2)]
            kT = [persist.tile([128, S], F32R, tag=f"kT{p}", name=f"kT{p}") for p in range(2)]
            v_sb = [persist.tile([128, HPC, 65], F32R, tag=f"v{s}", name=f"v{s}") for s in range(SC)]
            out_sb = [persist.tile([128, DPC], F32, tag=f"o{s}", name=f"o{s}") for s in range(SC)]

            # ---------------- phase A+B: transposes + projections -----------
            with (
                tc.tile_pool(name="proj", bufs=1) as projp,
                tc.tile_pool(name="hidp", bufs=2) as hidp,
                tc.tile_pool(name="hidTp", bufs=2) as hidTp,
                tc.tile_pool(name="trps", bufs=2, space="PSUM") as trps,
                tc.tile_pool(name="projps", bufs=2, space="PSUM") as projps,
                tc.tile_pool(name="vps", bufs=2, space="PSUM") as vps,
            ):
                w_sb = {}
                for name, w in (("wq", wq), ("wk", wk), ("wv", wv)):
                    t = projp.tile([128, JC, DPC], F32R, tag=name, name=f"w_{name}")
                    nc.sync.dma_start(
                        t[:], w.rearrange("(c p) n -> p c n", p=128).bitcast(F32R)
                    )
                    w_sb[name] = t

                for sblk in range(4):
                    hid_t = [hidp.tile([128, H], F32, tag=f"hid{i}", name=f"hid{i}") for i in range(4)]
                    for i in range(4):
                        s = sblk * 4 + i
                        nc.sync.dma_start(hid_t[i][:], hid[s * 128 : (s + 1) * 128, :])
                    hT = [
                        hidTp.tile([128, 512], F32R, tag=f"hidT{j}", name=f"hidT{j}") for j in range(JC)
                    ]
                    for j in range(JC):
                        tps = trps.tile([128, 512], F32, tag="trps", name="trps_t")
                        for i in range(4):
                            nc.tensor.transpose(
                                tps[:, i * 128 : (i + 1) * 128],
                                hid_t[i][:, j * 128 : (j + 1) * 128],
                                ident[:],
                            )
                        nc.vector.tensor_copy(hT[j][:], tps[:])
                    # qT / kT projections for this 512-wide s block
                    for wname, dst, bias in (("wq", qT, bqs_sb), ("wk", kT, bks_sb)):
                        for p in range(2):
                            ps = projps.tile([128, 512], F32, tag="projps", name="projps_t")
                            for j in range(JC):
                                nc.tensor.matmul(
                                    ps[:],
                                    w_sb[wname][:, j, p * 128 : (p + 1) * 128],
                                    hT[j][:],
                                    start=(j == 0),
                                    stop=(j == JC - 1),
                                )
                            nc.vector.tensor_scalar_add(
                                dst[p][:, sblk * 512 : (sblk + 1) * 512],
                                ps[:],
                                bias[:, p : p + 1],
                            )
                    # V projection for the 4 s chunks of this block
                    for i in range(4):
                        s = sblk * 4 + i
                        ps = vps.tile([128, DPC], F32, tag="vps", name="vps_t")
                        for j in range(JC):
                            nc.tensor.matmul(
                                ps[:],
                                hT[j][:, i * 128 : (i + 1) * 128],
                                w_sb["wv"][:, j, :],
                                start=(j == 0),
                                stop=False,
                            )
                        nc.tensor.matmul(
                            ps[:], ones1[:], bvs_sb[:], start=False, stop=True
                        )
                        ps3 = ps.rearrange("p (h c) -> p h c", h=HPC)
                        nc.vector.tensor_copy(v_sb[s][:, :, 0:HD], ps3[:])
                        nc.vector.tensor_copy(
                            v_sb[s][:, :, HD : HD + 1],
                            ones4_f[:].rearrange("p (h o) -> p h o", o=1),
                        )

            # ---------------- phase C+D: attention ---------------------------
            with (
                tc.tile_pool(name="etp", bufs=3) as etp,
                tc.tile_pool(name="ctsp", bufs=2) as ctsp,
                tc.tile_pool(name="rcp", bufs=4) as rcp,
                tc.tile_pool(name="scps", bufs=2, space="PSUM") as scps,
                tc.tile_pool(name="ctxps", bufs=1, space="PSUM") as ctxps,
                tc.tile_pool(name="dps", bufs=2, space="PSUM") as dps,
            ):
                for pair in range(2):
                    h0, h1 = 2 * pair, 2 * pair + 1
                    for qb in range(NQB):
                        qs = qb * QB
                        ctx0 = ctxps.tile([65, QB], F32, tag="ctx0", name="ctx0")
                        ctx1 = ctxps.tile([65, QB], F32, tag="ctx1", name="ctx1")
                        for k in range(SC):
                            st = scps.tile([128, 2 * QB], F32, tag="sc", name="sc_t")
                            nc.tensor.matmul(
                                st[:, 0:QB],
                                kT[pair][0:64, k * 128 : (k + 1) * 128],
                                qT[pair][0:64, qs : qs + QB],
                                start=True,
                                stop=True,
                            )
                            nc.tensor.matmul(
                                st[:, QB : 2 * QB],
                                kT[pair][64:128, k * 128 : (k + 1) * 128],
                                qT[pair][64:128, qs : qs + QB],
                                start=True,
                                stop=True,
                            )
                            et = etp.tile([128, 2 * QB], F32R, tag="et", name="et_t")
                            nc.scalar.activation(
                                et[:], st[:], EXP, bias=mask_sb[:, k : k + 1], scale=1.0
                            )
                            nc.tensor.matmul(
                                ctx0[:],
                                v_sb[k][:, h0, :],
                                et[:, 0:QB],
                                start=(k == 0),
                                stop=(k == SC - 1),
                            )
                            nc.tensor.matmul(
                                ctx1[:],
                                v_sb[k][:, h1, :],
                                et[:, QB : 2 * QB],
                                start=(k == 0),
                                stop=(k == SC - 1),
                            )
                        for h, ctx in ((h0, ctx0), (h1, ctx1)):
                            cts = ctsp.tile([65, QB], F32, tag="cts", name="cts_t")
                            nc.vector.tensor_copy(cts[:], ctx[:])
                            for i in range(QB // 128):
                                tp = dps.tile([128, 65], F32, tag="dps", name="dps_t")
                                nc.tensor.transpose(
                                    tp[:],
                                    cts[:, i * 128 : (i + 1) * 128],
                                    ident[0:65, 0:65],
                                )
                                rc = rcp.tile([128, 1], F32, tag="rc", name="rc_t")
                                nc.vector.reciprocal(rc[:], tp[:, HD : HD + 1])
                                qc = qb * (QB // 128) + i
                                nc.vector.tensor_scalar_mul(
                                    out_sb[qc][:, h * HD : (h + 1) * HD],
                                    tp[:, 0:HD],
                                    rc[:],
                                )
            for s in range(SC):
                nc.sync.dma_start(out[s * 128 : (s + 1) * 128, :], out_sb[s][:])

    nc.compile()
    return nc


def make_in_maps(hidden_states, attention_mask, Wq, bq, Wk, bk, Wv, bv):
    hidden_states = np.asarray(hidden_states, dtype=np.float32)
    attention_mask = np.asarray(attention_mask, dtype=np.float32)
    Wq = np.asarray(Wq, dtype=np.float32)
    bq = np.asarray(bq, dtype=np.float32)
    Wk = np.asarray(Wk, dtype=np.float32)
    bk = np.asarray(bk, dtype=np.float32)
    Wv = np.asarray(Wv, dtype=np.float32)
    bv = np.asarray(bv, dtype=np.float32)

    in_maps = []
    for c in range(NCORES):
        b = c // 4
        g = c % 4
        rows = slice(g * DPC, (g + 1) * DPC)
        wq_c = np.ascontiguousarray((Wq[rows, :] * 0.125).T)  # [H, DPC]
        wk_c = np.ascontiguousarray(Wk[rows, :].T)
        wv_c = np.ascontiguousarray(Wv[rows, :].T)
        bqs = np.ascontiguousarray((bq[rows] * 0.125).reshape(2, 128).T)  # [128, 2]
        bks = np.ascontiguousarray(bk[rows].reshape(2, 128).T)
        bvs = np.ascontiguousarray(bv[rows].reshape(1, DPC))
        mask_c = np.ascontiguousarray(
            attention_mask[b, 0, 0, :].reshape(SC, 128).T
        )  # [128, SC]
        in_maps.append(
            {
                "hid": np.ascontiguousarray(hidden_states[b]),
                "wq": wq_c,
                "wk": wk_c,
                "wv": wv_c,
                "bqs": bqs,
                "bks": bks,
                "bvs": bvs,
                "mask": mask_c,
            }
        )
    return in_maps


def gather(results):
    full = np.empty((B, S, H), dtype=np.float32)
    for c in range(NCORES):
        b = c // 4
        g = c % 4
        full[b, :, g * DPC : (g + 1) * DPC] = results[c]["out"]
    return full


_NC = None


def kernel(hidden_states, attention_mask, Wq, bq, Wk, bk, Wv, bv, **run_kwargs):
    global _NC
    if _NC is None:
        _NC = build()
    in_maps = make_in_maps(hidden_states, attention_mask, Wq, bq, Wk, bk, Wv, bv)
    res = run_bass_kernel_spmd(_NC, in_maps, core_ids=list(range(NCORES)), **run_kwargs)
    out = gather(res.results)
    if run_kwargs:
        kernel.last_result = res
    return out


# revision 7
# speedup vs baseline: 1.0354x; 1.0354x over previous
"""BertSelfAttention (B=2, S=2048, H=1024, 16 heads x 64) on 8 TRN2 NeuronCores.

Sharding: data parallel on batch (4 cores per batch) x tensor parallel on
heads (4 heads per core). No cross-core comms; each core computes
out[b, :, 256*g:256*(g+1)] for its head group g.

v2: bf16 matmul operands everywhere (enables FWL / pipelined LDWEIGHTS; host
pre-casts hidden+weights to bf16), hiddenT via DMA transpose (X-bar),
attention inner loop with N=1024 moving operands, one head at a time.

Per-core pipeline:
  A) hiddenT [128(j), 2048(s)] bf16 tiles via 8 transposing DMAs
  B) qT/kT [128(d of head pair), 2048(s)] bf16 projections (1/8 scale and
     bias folded), V [128(s), 4heads, 65] bf16 with a ones column per head
     (softmax denominator comes out of the ctx matmul for free)
  C) per head: scoresT[k,q] = kT.T @ qT (K=64, N=1024), exp on ACT
     (mask as per-partition bias) -> bf16, ctxT[65, q] += v_ext.T @ expT
     accumulated over 16 k chunks in PSUM
  D) PE-transpose ctxT -> [q, 65], DVE reciprocal + scale, DMA out (fp32)
"""

import ml_dtypes
import numpy as np

import concourse.bass as bass
import concourse.tile as tile
from concourse import bacc, mybir
from concourse.bass_utils import run_bass_kernel_spmd
from concourse.masks import make_identity

F32 = mybir.dt.float32
BF16 = mybir.dt.bfloat16
EXP = mybir.ActivationFunctionType.Exp

B, S, H = 2, 2048, 1024
NH, HD = 16, 64
NCORES = 8
HPC = 4  # heads per core
DPC = HPC * HD  # 256 output dims per core
SC = S // 128  # 16 s/k chunks
JC = H // 128  # 8 contraction chunks
QH = 1024  # q block in attention inner loop
NQH = S // QH


def build():
    nc = bacc.Bacc(
        "TRN2",
        target_bir_lowering=False,
        debug=False,
        enable_asserts=False,
        num_devices=NCORES,
    )
    hidb = nc.dram_tensor("hidb", [S, H], BF16, kind="ExternalInput").ap()
    wq = nc.dram_tensor("wq", [H, DPC], BF16, kind="ExternalInput").ap()
    wk = nc.dram_tensor("wk", [H, DPC], BF16, kind="ExternalInput").ap()
    wv = nc.dram_tensor("wv", [H, DPC], BF16, kind="ExternalInput").ap()
    bqs = nc.dram_tensor("bqs", [128, 2], F32, kind="ExternalInput").ap()
    bks = nc.dram_tensor("bks", [128, 2], F32, kind="ExternalInput").ap()
    bvs = nc.dram_tensor("bvs", [1, DPC], BF16, kind="ExternalInput").ap()
    mask = nc.dram_tensor("mask", [128, SC], F32, kind="ExternalInput").ap()
    out = nc.dram_tensor("out", [S, DPC], F32, kind="ExternalOutput").ap()

    with tile.TileContext(nc) as tc:
        with tc.tile_pool(name="persist", bufs=1) as persist:
            ident = persist.tile([128, 128], F32, tag="ident")
            make_identity(nc, ident[:])
            ones1_f = persist.tile([1, 128], F32, tag="ones1f")
            nc.vector.memset(ones1_f[:], 1.0)
            ones1 = persist.tile([1, 128], BF16, tag="ones1")
            nc.vector.tensor_copy(ones1[:], ones1_f[:])
            ones4_f = persist.tile([128, HPC], F32, tag="ones4f")
            nc.vector.memset(ones4_f[:], 1.0)
            mask_sb = persist.tile([128, SC], F32, tag="mask")
            nc.sync.dma_start(mask_sb[:], mask)
            bqs_sb = persist.tile([128, 2], F32, tag="bqs")
            nc.sync.dma_start(bqs_sb[:], bqs)
            bks_sb = persist.tile([128, 2], F32, tag="bks")
            nc.sync.dma_start(bks_sb[:], bks)
            bvs_sb = persist.tile([1, DPC], BF16, tag="bvs")
            nc.sync.dma_start(bvs_sb[:], bvs)

            # persistent activations
            qT = [
                persist.tile([128, S], BF16, tag=f"qT{p}", name=f"qT{p}")
                for p in range(2)
            ]
            kT = [
                persist.tile([128, S], BF16, tag=f"kT{p}", name=f"kT{p}")
                for p in range(2)
            ]
            v_sb = [
                persist.tile([128, HPC, 65], BF16, tag=f"v{s}", name=f"v{s}")
                for s in range(SC)
            ]
            out_sb = [
                persist.tile([128, DPC], F32, tag=f"o{s}", name=f"o{s}")
                for s in range(SC)
            ]
            hidT = [
                persist.tile([128, S], BF16, tag=f"hidT{j}", name=f"hidT{j}")
                for j in range(JC)
            ]

            for j in range(JC):
                nc.sync.dma_start_transpose(
                    out=hidT[j][:], in_=hidb[:, j * 128 : (j + 1) * 128]
                )

            # ---------------- phase B: projections --------------------------
            with (
                tc.tile_pool(name="proj", bufs=1) as projp,
                tc.tile_pool(name="projps", bufs=2, space="PSUM") as projps,
                tc.tile_pool(name="vps", bufs=2, space="PSUM") as vps,
            ):
                w_sb = {}
                for name, w in (("wq", wq), ("wk", wk), ("wv", wv)):
                    t = projp.tile([128, JC, DPC], BF16, tag=name, name=f"w_{name}")
                    nc.sync.dma_start(t[:], w.rearrange("(c p) n -> p c n", p=128))
                    w_sb[name] = t

                for wname, dst, bias in (("wq", qT, bqs_sb), ("wk", kT, bks_sb)):
                    for p in range(2):
                        for half in range(2):
                            ps = projps.tile(
                                [128, QH], F32, tag="projps", name="projps_t"
                            )
                            for j in range(JC):
                                for q2 in range(2):
                                    nc.tensor.matmul(
                                        ps[:, q2 * 512 : (q2 + 1) * 512],
                                        w_sb[wname][:, j, p * 128 : (p + 1) * 128],
                                        hidT[j][
                                            :,
                                            half * QH + q2 * 512 : half * QH
                                            + (q2 + 1) * 512,
                                        ],
                                        start=(j == 0),
                                        stop=(j == JC - 1),
                                    )
                            nc.vector.tensor_scalar_add(
                                dst[p][:, half * QH : (half + 1) * QH],
                                ps[:],
                                bias[:, p : p + 1],
                            )
                for s in range(SC):
                    ps = vps.tile([128, DPC], F32, tag="vps", name="vps_t")
                    for j in range(JC):
                        nc.tensor.matmul(
                            ps[:],
                            hidT[j][:, s * 128 : (s + 1) * 128],
                            w_sb["wv"][:, j, :],
                            start=(j == 0),
                            stop=False,
                        )
                    nc.tensor.matmul(ps[:], ones1[:], bvs_sb[:], start=False, stop=True)
                    ps3 = ps.rearrange("p (h c) -> p h c", h=HPC)
                    nc.vector.tensor_copy(v_sb[s][:, :, 0:HD], ps3[:])
                    nc.vector.tensor_copy(
                        v_sb[s][:, :, HD : HD + 1],
                        ones4_f[:].rearrange("p (h o) -> p h o", o=1),
                    )

            # ---------------- phase C+D: attention ---------------------------
            with (
                tc.tile_pool(name="etp", bufs=3) as etp,
                tc.tile_pool(name="ctsp", bufs=2) as ctsp,
                tc.tile_pool(name="rcp", bufs=4) as rcp,
                tc.tile_pool(name="scps", bufs=2, space="PSUM") as scps,
                tc.tile_pool(name="ctxps", bufs=1, space="PSUM") as ctxps,
                tc.tile_pool(name="dps", bufs=2, space="PSUM") as dps,
            ):
                for h in range(HPC):
                    p, hp = h // 2, (h % 2) * 64
                    for qh in range(NQH):
                        qs = qh * QH
                        ctx = ctxps.tile([65, QH], F32, tag="ctx", name="ctx_t")
                        for k in range(SC):
                            st = scps.tile([128, QH], F32, tag="sc", name="sc_t")
                            for q2 in range(2):
                                nc.tensor.matmul(
                                    st[:, q2 * 512 : (q2 + 1) * 512],
                                    kT[p][hp : hp + 64, k * 128 : (k + 1) * 128],
                                    qT[p][hp : hp + 64, qs + q2 * 512 : qs + (q2 + 1) * 512],
                                    start=True,
                                    stop=True,
                                )
                            et = etp.tile([128, QH], BF16, tag="et", name="et_t")
                            nc.scalar.activation(
                                et[:], st[:], EXP, bias=mask_sb[:, k : k + 1], scale=1.0
                            )
                            for q2 in range(2):
                                nc.tensor.matmul(
                                    ctx[:, q2 * 512 : (q2 + 1) * 512],
                                    v_sb[k][:, h, :],
                                    et[:, q2 * 512 : (q2 + 1) * 512],
                                    start=(k == 0),
                                    stop=(k == SC - 1),
                                )
                        cts = ctsp.tile([65, QH], F32, tag="cts", name="cts_t")
                        nc.vector.tensor_copy(cts[:], ctx[:])
                        for i in range(QH // 128):
                            tp = dps.tile([128, 65], F32, tag="dps", name="dps_t")
                            nc.tensor.transpose(
                                tp[:],
                                cts[:, i * 128 : (i + 1) * 128],
                                ident[0:65, 0:65],
                            )
                            rc = rcp.tile([128, 1], F32, tag="rc", name="rc_t")
                            nc.vector.reciprocal(rc[:], tp[:, HD : HD + 1])
                            qc = qh * (QH // 128) + i
                            nc.vector.tensor_scalar_mul(
                                out_sb[qc][:, h * HD : (h + 1) * HD],
                                tp[:, 0:HD],
                                rc[:],
                            )
            for s in range(SC):
                nc.sync.dma_start(out[s * 128 : (s + 1) * 128, :], out_sb[s][:])

    nc.compile()
    return nc


def make_in_maps(hidden_states, attention_mask, Wq, bq, Wk, bk, Wv, bv):
    hidden_states = np.asarray(hidden_states, dtype=np.float32)
    attention_mask = np.asarray(attention_mask, dtype=np.float32)
    Wq = np.asarray(Wq, dtype=np.float32)
    bq = np.asarray(bq, dtype=np.float32)
    Wk = np.asarray(Wk, dtype=np.float32)
    bk = np.asarray(bk, dtype=np.float32)
    Wv = np.asarray(Wv, dtype=np.float32)
    bv = np.asarray(bv, dtype=np.float32)
    bf = ml_dtypes.bfloat16

    in_maps = []
    for c in range(NCORES):
        b = c // 4
        g = c % 4
        rows = slice(g * DPC, (g + 1) * DPC)
        in_maps.append(
            {
                "hidb": np.ascontiguousarray(hidden_states[b]).astype(bf),
                "wq": np.ascontiguousarray((Wq[rows, :] * 0.125).T).astype(bf),
                "wk": np.ascontiguousarray(Wk[rows, :].T).astype(bf),
                "wv": np.ascontiguousarray(Wv[rows, :].T).astype(bf),
                "bqs": np.ascontiguousarray((bq[rows] * 0.125).reshape(2, 128).T),
                "bks": np.ascontiguousarray(bk[rows].reshape(2, 128).T),
                "bvs": np.ascontiguousarray(bv[rows].reshape(1, DPC)).astype(bf),
                "mask": np.ascontiguousarray(attention_mask[b, 0, 0, :].reshape(SC, 128).T),
            }
        )
    return in_maps


def gather(results):
    full = np.empty((B, S, H), dtype=np.float32)
    for c in range(NCORES):
        b = c // 4
        g = c % 4
        full[b, :, g * DPC : (g + 1) * DPC] = results[c]["out"]
    return full


_NC = None


def kernel(hidden_states, attention_mask, Wq, bq, Wk, bk, Wv, bv, **run_kwargs):
    global _NC
    if _NC is None:
        _NC = build()
    in_maps = make_in_maps(hidden_states, attention_mask, Wq, bq, Wk, bk, Wv, bv)
    res = run_bass_kernel_spmd(_NC, in_maps, core_ids=list(range(NCORES)), **run_kwargs)
    out = gather(res.results)
    if run_kwargs:
        kernel.last_result = res
    return out
